# revision 6
# baseline (speedup 1.0000x reference)
"""AdaptiveGraphConv (Chebyshev K=3 graph conv) on 8 TRN2 NeuronCores.

Row-sharded over the 4096 nodes: core k owns nodes [512k, 512(k+1)).
 - adj is binary+symmetric, so core k's lhsT (= A[:, shard_k]) is loaded once,
   cast to bf16 (exact for 0/1), and kept in SBUF for both Laplacian matmuls.
 - degrees: on-device partial row sums + ReduceScatter.
 - channel mixes (W0-W2, W1, W2) done at entry in the natural [(b,c), (n,t)]
   layout via a block-diagonal 128x128 weight, since the channel mix commutes
   with the graph operator L.
 - normalization s = d^-1/2 applied as per-partition scales in node-major
   layout; two AllGathers (bf16) move the matmul rhs operands between cores.

Math (S = diag(s), A binary adj, L = I - S A S):
  out = h(W0-W2) + (Lh)W1 + 2 L(L h) W2 + bias
      = P0 + L M,   M = P1 + 2(P2 - S Z2),  Z2 = A(S P2)
  out = P0 + M - S Z3,  Z3 = A (S M)
with P0 = h(W0-W2)+bias, P1 = h W1, P2 = h W2.
"""

from contextlib import ExitStack

import numpy as np

import concourse.bacc as bacc
import concourse.mybir as mybir
import concourse.tile as tile
from concourse.bass_utils import run_bass_kernel_spmd
from concourse.masks import make_identity

P = 128
NCORES = 8
N = 4096
S = N // NCORES          # 512 nodes per core
B, C, T = 4, 32, 12
F = B * C * T            # 1536 flattened (b, o, t) columns, f = 384b + 12o + t
NT = S * T               # 6144 free columns in (b,c)-major layout
KT = N // P              # 32 contraction tiles
MJ = S // P              # 4 node tiles per core
FB = 512                 # matmul moving-free block
NFB = F // FB            # 3
KH = KT // 2             # K-half for streaming the gathered rhs

f32 = mybir.dt.float32
bf16 = mybir.dt.bfloat16
AX = mybir.AxisListType
ALU = mybir.AluOpType

_CACHE = {}


def _graph_kernel(ctx, tc, xs, adjT, w, bias, out):
    nc = tc.nc
    RG = [list(range(NCORES))]

    consts = ctx.enter_context(tc.tile_pool(name="consts", bufs=1))
    persist = ctx.enter_context(tc.tile_pool(name="persist", bufs=1))
    scratch = ctx.enter_context(tc.tile_pool(name="scratch", bufs=3))
    stream = ctx.enter_context(tc.tile_pool(name="stream", bufs=3))
    psum = ctx.enter_context(tc.tile_pool(name="psum", bufs=2, space="PSUM"))
    dram = ctx.enter_context(tc.tile_pool(name="dram", bufs=1, space="DRAM"))

    # ---------------- Phase A: adjacency load + cast + partial degrees
    abf = persist.tile([P, KT, S], bf16)      # lhsT tiles, kept resident
    dpart = consts.tile([P, KT], f32)
    for ki in range(KT):
        af = stream.tile([P, S], f32, tag="af", name=f"af{ki}")
        nc.sync.dma_start(af[:], adjT[P * ki:P * (ki + 1), :])
        nc.vector.tensor_copy(abf[:, ki, :], af[:])
        nc.vector.reduce_sum(dpart[:, ki:ki + 1], af[:], axis=AX.X)

    # degrees: ReduceScatter -> each core gets d for its own shard
    d_in = dram.tile([KT, P], f32)
    d_out = dram.tile([MJ, P], f32)
    nc.sync.dma_start(d_in.rearrange("i p -> p i"), dpart[:])
    nc.gpsimd.collective_compute(
        "ReduceScatter", ALU.add, replica_groups=RG,
        ins=[d_in.opt()], outs=[d_out.opt()],
    )
    s_raw = consts.tile([P, MJ], f32)
    nc.sync.dma_start(s_raw[:], d_out.rearrange("a p -> p a"))
    # s = sqrt(1/max(d, 0.5)) * min(d, 1)   (d is an exact non-negative integer)
    s_dc = consts.tile([P, MJ], f32)
    nc.vector.tensor_scalar_max(s_dc[:], s_raw[:], 0.5)
    s_r = consts.tile([P, MJ], f32)
    nc.vector.reciprocal(s_r[:], s_dc[:])
    s_q = consts.tile([P, MJ], f32)
    nc.scalar.activation(s_q[:], s_r[:], mybir.ActivationFunctionType.Sqrt)
    s_m = consts.tile([P, MJ], f32)
    nc.vector.tensor_scalar_min(s_m[:], s_raw[:], 1.0)
    s_t = consts.tile([P, MJ], f32)
    nc.vector.tensor_tensor(s_t[:], s_q[:], s_m[:], op=ALU.mult)
    sm2 = consts.tile([P, MJ], f32)   # -2s
    nc.vector.tensor_scalar_mul(sm2[:], s_t[:], -2.0)
    smn = consts.tile([P, MJ], f32)   # -s
    nc.vector.tensor_scalar_mul(smn[:], s_t[:], -1.0)

    # ---------------- constants: block-diag weights, bias, identity
    wblk = []
    for j in range(3):
        wb = consts.tile([P, P], f32, name=f"wblk{j}")
        nc.vector.memset(wb[:], 0.0)
        for b in range(B):
            nc.sync.dma_start(wb[32 * b:32 * (b + 1), 32 * b:32 * (b + 1)], w[j])
        wblk.append(wb)
    wd = consts.tile([P, P], f32)
    nc.vector.tensor_tensor(wd[:], wblk[0][:], wblk[2][:], op=ALU.subtract)
    brep = consts.tile([P, 1], f32)
    bias_v = bias.rearrange("(c o) -> c o", o=1)
    for b in range(B):
        nc.sync.dma_start(brep[32 * b:32 * (b + 1), :], bias_v)
    ident = consts.tile([P, P], f32)
    make_identity(nc, ident[:])

    # ---------------- Phase B: entry — load x, channel mixes, transposes
    xt = scratch.tile([P, NT], f32, tag="sc", name="xt")
    nc.sync.dma_start(xt[:], xs[:])
    p0 = persist.tile([P, NT], f32)           # (b,o)-major, until exit
    p1bo = scratch.tile([P, NT], f32, tag="sc", name="p1bo")
    p2bo = scratch.tile([P, NT], f32, tag="sc", name="p2bo")
    for lhs, dest, nm in ((wd, p0, "m0"), (wblk[1], p1bo, "m1"), (wblk[2], p2bo, "m2")):
        for fi in range(NT // FB):
            pmix = psum.tile([P, FB], f32, tag="pmix", bufs=2, name=f"pmix_{nm}_{fi}")
            nc.tensor.matmul(pmix[:], lhs[:], xt[:, FB * fi:FB * (fi + 1)],
                             start=True, stop=True)
            nc.vector.tensor_copy(dest[:, FB * fi:FB * (fi + 1)], pmix[:])
    nc.vector.tensor_scalar_add(p0[:], p0[:], brep[:])   # bias (zeros in practice)

    # node-major tensors: [p, mj, f] with n_local = 128*mj + p
    p1n = persist.tile([P, MJ, F], f32)       # becomes M, then out_n (in place)
    p2n = persist.tile([P, MJ, F], bf16)
    ustage = persist.tile([P, MJ, F], bf16)   # AllGather staging (scaled, bf16)
    p1n_v = p1n.rearrange("p m (o t) -> p m t o", t=T)
    p2n_v = p2n.rearrange("p m (o t) -> p m t o", t=T)
    ustage_v = ustage.rearrange("p m (o t) -> p m t o", t=T)
    for src, kind in ((p1bo, 1), (p2bo, 2)):
        src_v = src.rearrange("p (n t) -> p n t", t=T)
        for mj in range(MJ):
            for t in range(T):
                pt = psum.tile([P, P], f32, tag="pt", bufs=2,
                               name=f"pt_{kind}_{mj}_{t}")
                nc.tensor.transpose(pt[:], src_v[:, P * mj:P * (mj + 1), t], ident[:])
                if kind == 1:
                    nc.vector.tensor_copy(p1n_v[:, mj, t, :], pt[:])
                else:
                    nc.vector.tensor_copy(p2n_v[:, mj, t, :], pt[:])
                    nc.vector.tensor_scalar_mul(
                        ustage_v[:, mj, t, :], pt[:], s_t[:, mj:mj + 1])

    # ---------------- AllGather 1: s*P2 -> full rhs
    ag1_in = dram.tile([S, F], bf16)
    ushr1 = dram.tile([N, F], bf16, addr_space="Shared")
    nc.sync.dma_start(ag1_in.rearrange("(m p) f -> p m f", p=P), ustage[:])
    nc.gpsimd.collective_compute(
        "AllGather", ALU.bypass, replica_groups=RG,
        ins=[ag1_in.opt()], outs=[ushr1.opt()],
    )
    ushr1_v = ushr1.rearrange("(ki p) f -> p ki f", p=P)

    # ---------------- MM1: Z2 = A (s*P2); M = P1 + 2*P2 - 2*s*Z2 (in p1n)
    for fi in range(NFB):
        uh = []
        for h in range(2):
            u = scratch.tile([P, KH, FB], bf16, tag="sc", name=f"u1_{fi}_{h}")
            nc.sync.dma_start(
                u[:], ushr1_v[:, KH * h:KH * (h + 1), FB * fi:FB * (fi + 1)])
            uh.append(u)
        for mj in range(MJ):
            pm = psum.tile([P, FB], f32, tag="pm", bufs=4, name=f"pm1_{fi}_{mj}")
            for ki in range(KT):
                nc.tensor.matmul(
                    pm[:],
                    abf[:, ki, P * mj:P * (mj + 1)],
                    uh[ki // KH][:, ki % KH, :],
                    start=(ki == 0), stop=(ki == KT - 1),
                )
            fsl = slice(FB * fi, FB * (fi + 1))
            # M = (Z2 * -2s) + P1 ; then M += 2*P2
            nc.vector.scalar_tensor_tensor(
                p1n[:, mj, fsl], pm[:], sm2[:, mj:mj + 1], p1n[:, mj, fsl],
                op0=ALU.mult, op1=ALU.add)
            nc.vector.scalar_tensor_tensor(
                p1n[:, mj, fsl], p2n[:, mj, fsl], 2.0, p1n[:, mj, fsl],
                op0=ALU.mult, op1=ALU.add)
            nc.vector.tensor_scalar_mul(
                ustage[:, mj, fsl], p1n[:, mj, fsl], s_t[:, mj:mj + 1])

    # ---------------- AllGather 2: s*M
    ag2_in = dram.tile([S, F], bf16)
    ushr2 = dram.tile([N, F], bf16, addr_space="Shared")
    nc.sync.dma_start(ag2_in.rearrange("(m p) f -> p m f", p=P), ustage[:])
    nc.gpsimd.collective_compute(
        "AllGather", ALU.bypass, replica_groups=RG,
        ins=[ag2_in.opt()], outs=[ushr2.opt()],
    )
    ushr2_v = ushr2.rearrange("(ki p) f -> p ki f", p=P)

    # ---------------- MM2: Z3 = A (s*M); out_n = M - s*Z3 (in p1n)
    for fi in range(NFB):
        uh = []
        for h in range(2):
            u = scratch.tile([P, KH, FB], bf16, tag="sc", name=f"u2_{fi}_{h}")
            nc.sync.dma_start(
                u[:], ushr2_v[:, KH * h:KH * (h + 1), FB * fi:FB * (fi + 1)])
            uh.append(u)
        for mj in range(MJ):
            pm = psum.tile([P, FB], f32, tag="pm", bufs=4, name=f"pm2_{fi}_{mj}")
            for ki in range(KT):
                nc.tensor.matmul(
                    pm[:],
                    abf[:, ki, P * mj:P * (mj + 1)],
                    uh[ki // KH][:, ki % KH, :],
                    start=(ki == 0), stop=(ki == KT - 1),
                )
            fsl = slice(FB * fi, FB * (fi + 1))
            nc.vector.scalar_tensor_tensor(
                p1n[:, mj, fsl], pm[:], smn[:, mj:mj + 1], p1n[:, mj, fsl],
                op0=ALU.mult, op1=ALU.add)

    # ---------------- exit: transpose back to (b,o)-major, add P0, store
    out_sb = scratch.tile([P, NT], f32, tag="sc", name="out_sb")
    out_v = out_sb.rearrange("p (n t) -> p n t", t=T)
    p0_v = p0.rearrange("p (n t) -> p n t", t=T)
    for mj in range(MJ):
        for t in range(T):
            pt = psum.tile([P, P], f32, tag="pt", bufs=2, name=f"pte_{mj}_{t}")
            nc.tensor.transpose(pt[:], p1n_v[:, mj, t, :], ident[:])
            nc.vector.tensor_tensor(
                out_v[:, P * mj:P * (mj + 1), t], pt[:],
                p0_v[:, P * mj:P * (mj + 1), t], op=ALU.add)
    nc.sync.dma_start(out[:], out_sb[:])


def build_nc():
    nc = bacc.Bacc(target_bir_lowering=False)
    xs = nc.declare_dram_parameter("xs", [P, NT], f32, isOutput=False)
    adjT = nc.declare_dram_parameter("adjT", [N, S], f32, isOutput=False)
    w = nc.declare_dram_parameter("w", [3, C, C], f32, isOutput=False)
    bias = nc.declare_dram_parameter("bias", [C], f32, isOutput=False)
    out = nc.declare_dram_parameter("out", [P, NT], f32, isOutput=True)
    with tile.TileContext(nc) as tc, ExitStack() as ctx:
        _graph_kernel(ctx, tc, xs, adjT, w, bias, out)
    nc.compile()
    return nc


def make_in_maps(x, adj, weight, bias):
    in_maps = []
    for k in range(NCORES):
        sl = slice(S * k, S * (k + 1))
        in_maps.append({
            "xs": np.ascontiguousarray(x[:, :, sl, :]).reshape(P, NT),
            "adjT": np.ascontiguousarray(adj[:, sl]),
            "w": np.ascontiguousarray(weight),
            "bias": np.ascontiguousarray(bias),
        })
    return in_maps


def kernel(x, adj, weight, bias, _trace=False, _tmpdir=None):
    if "nc" not in _CACHE:
        _CACHE["nc"] = build_nc()
    nc = _CACHE["nc"]
    in_maps = make_in_maps(
        np.asarray(x, np.float32), np.asarray(adj, np.float32),
        np.asarray(weight, np.float32), np.asarray(bias, np.float32))
    res = run_bass_kernel_spmd(nc, in_maps, core_ids=list(range(NCORES)),
                               trace=_trace, tmpdir=_tmpdir)
    _CACHE["last_result"] = res
    parts = [r["out"].reshape(B, C, S, T) for r in res.results]
    return np.concatenate(parts, axis=2)


# revision 14
# speedup vs baseline: 1.1289x; 1.1289x over previous
"""AdaptiveGraphConv (Chebyshev K=3 graph conv) on 8 TRN2 NeuronCores.

Row-sharded over the 4096 nodes: core k owns nodes [512k, 512(k+1)).
 - adj is binary+symmetric: core k's lhsT (= A[:, shard_k]) is loaded once,
   cast to bf16 (exact for 0/1) on GpSimd, and kept in SBUF for both
   Laplacian matmuls.
 - degrees: NO collective — d[m] for m in shard = column sums of the local
   adj slice (= row sums by symmetry), computed on the PE by accumulating
   matmuls against a ones vector.
 - channel mixes (W1, W2; W0-W2 at exit) in the natural [(b,c), (n,t)]
   layout via block-diagonal 128x128 weights (channel mix commutes with L).
 - normalization s = d^-1/2 applied as per-partition scales: on the staged
   AllGather payloads (s*P2, s*M) and on the matmul outputs (s[m]).
 - two AllGathers, each chunked into 4 per-mj collectives so comm overlaps
   compute.

Math (S = diag(s), A binary adj, L = I - S A S):
  out = h(W0-W2) + (Lh)W1 + 2 L(L h) W2 + bias
      = P0 + M - S Z3 where M = P1 + 2(P2 - S Z2),
        Z2 = A(S P2), Z3 = A(S M), P0 = h(W0-W2)+bias, Pj = h Wj.
"""

from contextlib import ExitStack

import numpy as np

import concourse.bacc as bacc
import concourse.mybir as mybir
import concourse.tile as tile
from concourse.bass_utils import run_bass_kernel_spmd
from concourse.masks import make_identity

P = 128
NCORES = 8
N = 4096
S = N // NCORES          # 512 nodes per core
B, C, T = 4, 32, 12
F = B * C * T            # 1536 flattened (b, o, t) columns, f = 384b + 12o + t
NT = S * T               # 6144 free columns in (b,c)-major layout
KT = N // P              # 32 contraction tiles
MJ = S // P              # 4 node tiles per core
FB = 512                 # matmul moving-free block
NFB = F // FB            # 3
KH = KT // 2             # 16: K-half for streaming the gathered rhs
NXC = NT // FB           # 12 x-chunks

f32 = mybir.dt.float32
bf16 = mybir.dt.bfloat16
AX = mybir.AxisListType
ALU = mybir.AluOpType
ACT_FN = mybir.ActivationFunctionType

_CACHE = {}


def _graph_kernel(ctx, tc, xs, adjT, w, bias, out):
    nc = tc.nc
    RG = [list(range(NCORES))]

    consts = ctx.enter_context(tc.tile_pool(name="consts", bufs=1))
    persist = ctx.enter_context(tc.tile_pool(name="persist", bufs=1))
    scratch = ctx.enter_context(tc.tile_pool(name="scratch", bufs=2))
    stream = ctx.enter_context(tc.tile_pool(name="stream", bufs=4))
    psum = ctx.enter_context(tc.tile_pool(name="psum", bufs=1, space="PSUM"))
    dram = ctx.enter_context(tc.tile_pool(name="dram", bufs=1, space="DRAM"))

    # ---------------- constants
    ones_col = consts.tile([P, 1], f32)
    nc.vector.memset(ones_col[:], 1.0)
    wblk = []
    for j in range(3):
        wb = consts.tile([P, P], f32, name=f"wblk{j}")
        nc.vector.memset(wb[:], 0.0)
        for b in range(B):
            nc.sync.dma_start(wb[32 * b:32 * (b + 1), 32 * b:32 * (b + 1)], w[j])
        wblk.append(wb)
    wd = consts.tile([P, P], f32)
    nc.vector.tensor_tensor(wd[:], wblk[0][:], wblk[2][:], op=ALU.subtract)
    brep = consts.tile([P, 1], f32)
    bias_v = bias.rearrange("(c o) -> c o", o=1)
    for b in range(B):
        nc.sync.dma_start(brep[32 * b:32 * (b + 1), :], bias_v)
    ident = consts.tile([P, P], f32)
    make_identity(nc, ident[:])
    identb = consts.tile([P, P], bf16)
    nc.gpsimd.tensor_copy(identb[:], ident[:])

    # ---------------- Phase A: adjacency load, bf16 cast (GpSimd), PE degrees
    abf = persist.tile([P, KT, S], bf16)      # lhsT tiles, resident all kernel
    pd = psum.tile([1, S], f32, tag="pd", bufs=1, name="pd")
    for ki in range(KT):
        af = stream.tile([P, S], f32, tag="af", name=f"af{ki}")
        nc.sync.dma_start(af[:], adjT[P * ki:P * (ki + 1), :])
        nc.gpsimd.tensor_copy(abf[:, ki, :], af[:])
        nc.tensor.matmul(pd[:], ones_col[:], af[:],
                         start=(ki == 0), stop=(ki == KT - 1))

    # d arrives free-major [1, 512]; bounce through DRAM to per-partition
    # [128, MJ] layout (tiny, off critical path)
    d_row = consts.tile([1, S], f32)
    nc.vector.tensor_copy(d_row[:], pd[:])
    d_dram = dram.tile([MJ, P], f32, name="d_dram")
    nc.sync.dma_start(d_dram.rearrange("a p -> (a p)").rearrange("(o s) -> o s", o=1),
                      d_row[:])
    # s = sqrt(1/max(d, 0.5)) * min(d, 1); d integral >= 0
    s_raw = consts.tile([P, MJ], f32)
    nc.sync.dma_start(s_raw[:], d_dram.rearrange("a p -> p a"))
    s_dc = consts.tile([P, MJ], f32)
    nc.vector.tensor_scalar_max(s_dc[:], s_raw[:], 0.5)
    s_r = consts.tile([P, MJ], f32)
    nc.vector.reciprocal(s_r[:], s_dc[:])
    s_q = consts.tile([P, MJ], f32)
    nc.scalar.activation(s_q[:], s_r[:], ACT_FN.Sqrt)
    s_m = consts.tile([P, MJ], f32)
    nc.vector.tensor_scalar_min(s_m[:], s_raw[:], 1.0)
    s_t = consts.tile([P, MJ], f32)
    nc.vector.tensor_tensor(s_t[:], s_q[:], s_m[:], op=ALU.mult)
    sm2 = consts.tile([P, MJ], f32)   # -2s
    nc.vector.tensor_scalar_mul(sm2[:], s_t[:], -2.0)
    smn = consts.tile([P, MJ], f32)   # -s
    nc.vector.tensor_scalar_mul(smn[:], s_t[:], -1.0)

    # ---------------- entry: stream x chunks, channel mixes P1 (f32), P2 (bf16)
    p1bo = scratch.tile([P, NT], f32, tag="sc", name="p1bo")
    p2bo = scratch.tile([P, NT], bf16, tag="sc", name="p2bo")
    for c in range(NXC):
        xc = stream.tile([P, FB], f32, tag="xc", name=f"xc{c}")
        nc.sync.dma_start(xc[:], xs[:, FB * c:FB * (c + 1)])
        csl = slice(FB * c, FB * (c + 1))
        pm2 = psum.tile([P, FB], f32, tag="pm", bufs=3, name=f"pm2_{c}")
        nc.tensor.matmul(pm2[:], wblk[2][:], xc[:], start=True, stop=True)
        nc.vector.tensor_copy(p2bo[:, csl], pm2[:])
        pm1 = psum.tile([P, FB], f32, tag="pm", bufs=3, name=f"pm1_{c}")
        nc.tensor.matmul(pm1[:], wblk[1][:], xc[:], start=True, stop=True)
        nc.vector.tensor_copy(p1bo[:, csl], pm1[:])

    # node-major tensors: [p, mj, f] with n_local = 128*mj + p
    p1n = persist.tile([P, MJ, F], f32)       # becomes M, then out_n (in place)
    p2n = persist.tile([P, MJ, F], bf16)
    ustage = persist.tile([P, MJ, F], bf16)   # AllGather staging (scaled bf16)
    p1n_v = p1n.rearrange("p m (o t) -> p m t o", t=T)
    p2n_v = p2n.rearrange("p m (o t) -> p m t o", t=T)
    ustage_v = ustage.rearrange("p m (o t) -> p m t o", t=T)
    p1bo_v = p1bo.rearrange("p (n t) -> p n t", t=T)
    p2bo_v = p2bo.rearrange("p (n t) -> p n t", t=T)

    # P2 transposes first (AG1 critical path), then AG1 chunk per mj
    ag1_out = []
    for mj in range(MJ):
        for t in range(T):
            pt = psum.tile([P, P], bf16, tag="pt", bufs=2, name=f"pt2_{mj}_{t}")
            nc.tensor.transpose(pt[:], p2bo_v[:, P * mj:P * (mj + 1), t], identb[:])
            nc.vector.tensor_scalar_mul(
                ustage_v[:, mj, t, :], pt[:], s_t[:, mj:mj + 1])
            # p2n holds 2*P2 (exact doubling) so the MM1 epilogue's GpSimd op
            # is a plain tensor_tensor add
            nc.scalar.activation(p2n_v[:, mj, t, :], pt[:], ACT_FN.Copy,
                                 scale=2.0)
        agi = dram.tile([P, F], bf16, name=f"ag1i{mj}")
        ago = dram.tile([NCORES * P, F], bf16, addr_space="Shared",
                        name=f"ag1o{mj}")
        nc.sync.dma_start(agi[:], ustage[:, mj, :])
        nc.gpsimd.collective_compute(
            "AllGather", ALU.bypass, replica_groups=RG,
            ins=[agi.opt()], outs=[ago.opt()],
        )
        ag1_out.append(ago)

    # P1 transposes: psum -> p1n (DVE copy; traced after AG1 so off the
    # AG critical path)
    for mj in range(MJ):
        for t in range(T):
            pt = psum.tile([P, P], f32, tag="pt1", bufs=1, name=f"pt1_{mj}_{t}")
            nc.tensor.transpose(pt[:], p1bo_v[:, P * mj:P * (mj + 1), t], ident[:])
            nc.vector.tensor_copy(p1n_v[:, mj, t, :], pt[:])

    def mm_pass(ag_bufs, tag, epilogue):
        # gathered rhs: global ki-tile = 4k + mjx lives in ag_bufs[mjx]
        # rows [128k, 128k+128). Stream K in halves of 16 ki-tiles.
        uh = []
        for h in range(2):
            u = scratch.tile([P, KH, F], bf16, tag="sc", name=f"u_{tag}_{h}")
            for mjx in range(MJ):
                src = ag_bufs[mjx].rearrange("(kq p) f -> p kq f", p=P)
                nc.sync.dma_start(u[:, mjx::MJ, :], src[:, 4 * h:4 * (h + 1), :])
            uh.append(u)
        for mj in range(MJ):
            pmf = []
            for fi in range(NFB):
                pm = psum.tile([P, FB], f32, tag="pm", bufs=3,
                               name=f"pmm_{tag}_{mj}_{fi}")
                pmf.append(pm)
            for ki in range(KT):
                lhs = abf[:, ki, P * mj:P * (mj + 1)]
                u = uh[ki // KH]
                kk = ki % KH
                for fi in range(NFB):
                    nc.tensor.matmul(
                        pmf[fi][:], lhs, u[:, kk, FB * fi:FB * (fi + 1)],
                        start=(ki == 0), stop=(ki == KT - 1))
            epilogue(mj, pmf)

    # ---------------- MM1: Z2 = A(s*P2); M = P1 + 2*P2 - 2*s*Z2 (in p1n)
    ag2_out = []

    def epi1(mj, pmf):
        for fi in range(NFB):
            fsl = slice(FB * fi, FB * (fi + 1))
            nc.vector.scalar_tensor_tensor(
                p1n[:, mj, fsl], pmf[fi][:], sm2[:, mj:mj + 1], p1n[:, mj, fsl],
                op0=ALU.mult, op1=ALU.add)
            nc.gpsimd.tensor_tensor(
                p1n[:, mj, fsl], p2n[:, mj, fsl], p1n[:, mj, fsl], op=ALU.add)
            nc.vector.tensor_scalar_mul(
                ustage[:, mj, fsl], p1n[:, mj, fsl], s_t[:, mj:mj + 1])
        agi = dram.tile([P, F], bf16, name=f"ag2i{mj}")
        ago = dram.tile([NCORES * P, F], bf16, addr_space="Shared",
                        name=f"ag2o{mj}")
        nc.sync.dma_start(agi[:], ustage[:, mj, :])
        nc.gpsimd.collective_compute(
            "AllGather", ALU.bypass, replica_groups=RG,
            ins=[agi.opt()], outs=[ago.opt()],
        )
        ag2_out.append(ago)

    mm_pass(ag1_out, "z2", epi1)

    # ---------------- MM2: Z3 = A(s*M); out_n = M - s*Z3 (in p1n)
    def epi2(mj, pmf):
        for fi in range(NFB):
            fsl = slice(FB * fi, FB * (fi + 1))
            nc.vector.scalar_tensor_tensor(
                p1n[:, mj, fsl], pmf[fi][:], smn[:, mj:mj + 1], p1n[:, mj, fsl],
                op0=ALU.mult, op1=ALU.add)

    mm_pass(ag2_out, "z3", epi2)

    # ---------------- exit: P0 = (W0-W2)-mix + bias into out_sb, then add
    # transposed node-major result, store.
    out_sb = scratch.tile([P, NT], f32, tag="sc", name="out_sb")
    for c in range(NXC):
        xc = stream.tile([P, FB], f32, tag="xc", name=f"xe{c}")
        nc.sync.dma_start(xc[:], xs[:, FB * c:FB * (c + 1)])
        pm0 = psum.tile([P, FB], f32, tag="pm", bufs=3, name=f"pm0_{c}")
        nc.tensor.matmul(pm0[:], wd[:], xc[:], start=True, stop=True)
        nc.scalar.activation(out_sb[:, FB * c:FB * (c + 1)], pm0[:],
                             ACT_FN.Identity, bias=brep[:, 0:1])
    out_v = out_sb.rearrange("p (n t) -> p n t", t=T)
    for mj in range(MJ):
        for t in range(T):
            pt = psum.tile([P, P], f32, tag="pt1", bufs=1, name=f"pte_{mj}_{t}")
            nc.tensor.transpose(pt[:], p1n_v[:, mj, t, :], ident[:])
            nc.vector.tensor_tensor(
                out_v[:, P * mj:P * (mj + 1), t], pt[:],
                out_v[:, P * mj:P * (mj + 1), t], op=ALU.add)
    nc.sync.dma_start(out[:], out_sb[:])


def build_nc():
    nc = bacc.Bacc(target_bir_lowering=False)
    xs = nc.declare_dram_parameter("xs", [P, NT], f32, isOutput=False)
    adjT = nc.declare_dram_parameter("adjT", [N, S], f32, isOutput=False)
    w = nc.declare_dram_parameter("w", [3, C, C], f32, isOutput=False)
    bias = nc.declare_dram_parameter("bias", [C], f32, isOutput=False)
    out = nc.declare_dram_parameter("out", [P, NT], f32, isOutput=True)
    with tile.TileContext(nc) as tc, ExitStack() as ctx:
        _graph_kernel(ctx, tc, xs, adjT, w, bias, out)
    nc.compile()
    return nc


def make_in_maps(x, adj, weight, bias):
    in_maps = []
    for k in range(NCORES):
        sl = slice(S * k, S * (k + 1))
        in_maps.append({
            "xs": np.ascontiguousarray(x[:, :, sl, :]).reshape(P, NT),
            "adjT": np.ascontiguousarray(adj[:, sl]),
            "w": np.ascontiguousarray(weight),
            "bias": np.ascontiguousarray(bias),
        })
    return in_maps


def kernel(x, adj, weight, bias, _trace=False, _tmpdir=None):
    if "nc" not in _CACHE:
        _CACHE["nc"] = build_nc()
    nc = _CACHE["nc"]
    in_maps = make_in_maps(
        np.asarray(x, np.float32), np.asarray(adj, np.float32),
        np.asarray(weight, np.float32), np.asarray(bias, np.float32))
    res = run_bass_kernel_spmd(nc, in_maps, core_ids=list(range(NCORES)),
                               trace=_trace, tmpdir=_tmpdir)
    _CACHE["last_result"] = res
    parts = [r["out"].reshape(B, C, S, T) for r in res.results]
    return np.concatenate(parts, axis=2)


# revision 17
# speedup vs baseline: 1.2815x; 1.1352x over previous
"""AdaptiveGraphConv (Chebyshev K=3 graph conv) on 8 TRN2 NeuronCores.

Row-sharded over the 4096 nodes: core k owns nodes [512k, 512(k+1)).
 - adj is binary+symmetric: core k's lhsT (= A[:, shard_k]) is loaded once
   (on the Activation HWDGE queue, so it doesn't block the x loads), cast to
   bf16 (exact for 0/1), resident in SBUF for both Laplacian matmuls.
 - degrees: no collective. d[m in shard] = column sums of the local adj slice
   (= row sums by symmetry) via PE matmul accumulation against ones.
 - host passes x with free layout (t, n): all three channel mixes computed as
   x_block^T @ W_blk (x stationary, block-diag weight moving), which lands
   node-major directly -> no entry transposes, P0 also node-major.
 - AllGathers chunked per mj (4 collectives per pass; first CC op ~22us,
   warm ones ~5us) with 4-phase matmul accumulation: phase ph consumes
   ki-tiles {4k+ph}, so matmuls start after the first chunk arrives.
 - MM loops: mj-pair outer (6 psum banks), phase/ki, fi inner.
 - exit (transpose back + bias) fused per-mj into the MM2 epilogue,
   streamed out in [128,128] blocks.

Math (S = diag(s), A binary adj, L = I - S A S):
  out = h(W0-W2) + (Lh)W1 + 2 L(L h) W2 + bias
      = P0 + M - S Z3;  M = P1 + 2(P2 - S Z2),
  Z2 = A(S P2), Z3 = A(S M), P0 = h(W0-W2), Pj = h Wj.
State: p1n (f32) holds P1 -> M -> out_n in place; p2n holds 2*P2 (bf16);
p0n holds P0 (bf16); ustage holds the scaled bf16 AG payloads.
"""

from contextlib import ExitStack

import numpy as np

import concourse.bacc as bacc
import concourse.mybir as mybir
import concourse.tile as tile
from concourse.bass_utils import run_bass_kernel_spmd
from concourse.masks import make_identity

P = 128
NCORES = 8
N = 4096
S = N // NCORES          # 512 nodes per core
B, C, T = 4, 32, 12
F = B * C * T            # 1536 flattened (t, bo) columns: f = 128*t + 32*b + o
NT = S * T               # 6144 free columns in (b,c)-major (t, n) layout
KT = N // P              # 32 contraction tiles
MJ = S // P              # 4 node tiles per core; also AG chunk / phase count
FB = 512                 # matmul moving-free block
NFB = F // FB            # 3
KPP = KT // MJ           # 8 ki-tiles per phase

f32 = mybir.dt.float32
bf16 = mybir.dt.bfloat16
ALU = mybir.AluOpType
ACT_FN = mybir.ActivationFunctionType

_CACHE = {}


def _graph_kernel(ctx, tc, xs, adjT, w, bias, out):
    nc = tc.nc
    RG = [list(range(NCORES))]

    consts = ctx.enter_context(tc.tile_pool(name="consts", bufs=1))
    persist = ctx.enter_context(tc.tile_pool(name="persist", bufs=1))
    scratch = ctx.enter_context(tc.tile_pool(name="scratch", bufs=4))
    stream = ctx.enter_context(tc.tile_pool(name="stream", bufs=4))
    psum = ctx.enter_context(tc.tile_pool(name="psum", bufs=1, space="PSUM"))
    dram = ctx.enter_context(tc.tile_pool(name="dram", bufs=1, space="DRAM"))

    # ---------------- constants
    ones_col = consts.tile([P, 1], f32)
    nc.vector.memset(ones_col[:], 1.0)
    wblk = []
    for j in range(3):
        wb = consts.tile([P, P], f32, name=f"wblk{j}")
        nc.vector.memset(wb[:], 0.0)
        for b in range(B):
            nc.sync.dma_start(wb[32 * b:32 * (b + 1), 32 * b:32 * (b + 1)], w[j])
        wblk.append(wb)
    wd = consts.tile([P, P], f32)
    nc.vector.tensor_tensor(wd[:], wblk[0][:], wblk[2][:], op=ALU.subtract)
    ident = consts.tile([P, P], f32)
    make_identity(nc, ident[:])
    brep = consts.tile([P, 1], f32)
    bias_v = bias.rearrange("(c o) -> c o", o=1)
    for b in range(B):
        nc.sync.dma_start(brep[32 * b:32 * (b + 1), :], bias_v)

    # ki-tile order: phase-major (MM consumes phase 0 = {ki % 4 == 0} first)
    ki_order = [MJ * k + ph for ph in range(MJ) for k in range(KPP)]

    # ---------------- Phase A: adjacency load (Activation HWDGE queue),
    # bf16 cast (GpSimd/DVE split), PE degree accumulation
    abf = persist.tile([P, KT, S], bf16)      # lhsT tiles, resident all kernel
    pd = psum.tile([1, S], f32, tag="pm", bufs=6, name="pd")
    for i, ki in enumerate(ki_order):
        af = stream.tile([P, S], f32, tag="af", bufs=3, name=f"af{ki}")
        nc.scalar.dma_start(af[:], adjT[P * ki:P * (ki + 1), :])
        if i % 2 == 0:
            nc.gpsimd.tensor_copy(abf[:, ki, :], af[:])
        else:
            nc.vector.tensor_copy(abf[:, ki, :], af[:])
        nc.tensor.matmul(pd[:], ones_col[:], af[:],
                         start=(i == 0), stop=(i == KT - 1))

    # d arrives free-major [1, 512]; bounce through DRAM to per-partition
    # [128, MJ] layout
    d_row = consts.tile([1, S], f32)
    nc.vector.tensor_copy(d_row[:], pd[:])
    d_dram = dram.tile([MJ, P], f32, name="d_dram")
    nc.sync.dma_start(
        d_dram.rearrange("a p -> (a p)").rearrange("(o s) -> o s", o=1), d_row[:])
    # s = sqrt(1/max(d, 0.5)) * min(d, 1); d integral >= 0
    s_raw = consts.tile([P, MJ], f32)
    nc.sync.dma_start(s_raw[:], d_dram.rearrange("a p -> p a"))
    s_dc = consts.tile([P, MJ], f32)
    nc.vector.tensor_scalar_max(s_dc[:], s_raw[:], 0.5)
    s_r = consts.tile([P, MJ], f32)
    nc.vector.reciprocal(s_r[:], s_dc[:])
    s_q = consts.tile([P, MJ], f32)
    nc.scalar.activation(s_q[:], s_r[:], ACT_FN.Sqrt)
    s_m = consts.tile([P, MJ], f32)
    nc.vector.tensor_scalar_min(s_m[:], s_raw[:], 1.0)
    s_t = consts.tile([P, MJ], f32)
    nc.vector.tensor_tensor(s_t[:], s_q[:], s_m[:], op=ALU.mult)
    sm2 = consts.tile([P, MJ], f32)   # -2s
    nc.vector.tensor_scalar_mul(sm2[:], s_t[:], -2.0)
    smn = consts.tile([P, MJ], f32)   # -s
    nc.vector.tensor_scalar_mul(smn[:], s_t[:], -1.0)

    # ---------------- node-major state: [p, mj, f], n_local = 128*mj + p,
    # f = 128*t + bo
    p1n = persist.tile([P, MJ, F], f32)       # P1 -> M -> out_n in place
    p2n = persist.tile([P, MJ, F], bf16)      # 2*P2
    p0n = persist.tile([P, MJ, F], bf16)      # P0
    ustage = persist.tile([P, MJ, F], bf16)   # AG staging (scaled bf16)

    # ---------------- entry: per (mj, t) block, three mixes land node-major
    # via x_block^T (stationary) @ W (moving); AG1 chunk fires per mj.
    ag1_out = [None] * MJ
    ag2_out = [None] * MJ
    for mj in range(MJ):
        for t in range(T):
            xcb = stream.tile([P, P], f32, tag="xcb", bufs=4,
                              name=f"xcb{mj}_{t}")
            nc.sync.dma_start(
                xcb[:], xs[:, FB * t + P * mj:FB * t + P * (mj + 1)])
            fsl = slice(P * t, P * (t + 1))
            ps2 = psum.tile([P, P], f32, tag="pe", bufs=2, name=f"ps2_{mj}_{t}")
            nc.tensor.matmul(ps2[:], xcb[:], wblk[2][:], start=True, stop=True)
            nc.vector.tensor_scalar_mul(
                ustage[:, mj, fsl], ps2[:], s_t[:, mj:mj + 1])
            nc.scalar.activation(p2n[:, mj, fsl], ps2[:], ACT_FN.Copy, scale=2.0)
            ps1 = psum.tile([P, P], f32, tag="pe", bufs=2, name=f"ps1_{mj}_{t}")
            nc.tensor.matmul(ps1[:], xcb[:], wblk[1][:], start=True, stop=True)
            nc.vector.tensor_copy(p1n[:, mj, fsl], ps1[:])
            ps0 = psum.tile([P, P], f32, tag="pe", bufs=2, name=f"ps0_{mj}_{t}")
            nc.tensor.matmul(ps0[:], xcb[:], wd[:], start=True, stop=True)
            nc.scalar.copy(p0n[:, mj, fsl], ps0[:])
        agi = dram.tile([P, F], bf16, name=f"ag1i{mj}")
        ago = dram.tile([NCORES * P, F], bf16, addr_space="Shared",
                        name=f"ag1o{mj}")
        nc.sync.dma_start(agi[:], ustage[:, mj, :])
        nc.gpsimd.collective_compute(
            "AllGather", ALU.bypass, replica_groups=RG,
            ins=[agi.opt()], outs=[ago.opt()],
        )
        ag1_out[mj] = ago

    def mm_pass(ag_bufs, tag, epilogue):
        # AG chunk ph rows: (k, p) -> global ki-tile 4k + ph, partition p.
        # 4 chunk tiles stay resident for both mj-pairs; accumulation is
        # 4-phase so matmuls start when chunk 0 lands.
        uhs = {}
        for pair_i, pair in enumerate(((0, 1), (2, 3))):
            pmf = {}
            for mj in pair:
                for fi in range(NFB):
                    pmf[(mj, fi)] = psum.tile(
                        [P, FB], f32, tag="pm", bufs=6,
                        name=f"pm_{tag}_{mj}_{fi}")
            for ph in range(MJ):
                if pair_i == 0:
                    uh = scratch.tile([P, KPP, F], bf16, tag="sc",
                                      name=f"uh_{tag}_{ph}")
                    nc.sync.dma_start(
                        uh[:], ag_bufs[ph].rearrange("(k p) f -> p k f", p=P))
                    uhs[ph] = uh
                uh = uhs[ph]
                for kk in range(KPP):
                    ki = MJ * kk + ph
                    for mj in pair:
                        lmj = abf[:, ki, P * mj:P * (mj + 1)]
                        for fi in range(NFB):
                            nc.tensor.matmul(
                                pmf[(mj, fi)][:], lmj,
                                uh[:, kk, FB * fi:FB * (fi + 1)],
                                start=(ph == 0 and kk == 0),
                                stop=(ph == MJ - 1 and kk == KPP - 1))
            for mj in pair:
                epilogue(mj, [pmf[(mj, fi)] for fi in range(NFB)])

    # ---------------- MM1: Z2 = A(s*P2); M = P1 + 2*P2 - 2*s*Z2 (in p1n)
    def epi1(mj, pmf):
        for fi in range(NFB):
            fsl = slice(FB * fi, FB * (fi + 1))
            nc.vector.scalar_tensor_tensor(
                p1n[:, mj, fsl], pmf[fi][:], sm2[:, mj:mj + 1], p1n[:, mj, fsl],
                op0=ALU.mult, op1=ALU.add)
            nc.gpsimd.tensor_tensor(
                p1n[:, mj, fsl], p2n[:, mj, fsl], p1n[:, mj, fsl], op=ALU.add)
            nc.vector.tensor_scalar_mul(
                ustage[:, mj, fsl], p1n[:, mj, fsl], s_t[:, mj:mj + 1])
        agi = dram.tile([P, F], bf16, name=f"ag2i{mj}")
        ago = dram.tile([NCORES * P, F], bf16, addr_space="Shared",
                        name=f"ag2o{mj}")
        nc.sync.dma_start(agi[:], ustage[:, mj, :])
        nc.gpsimd.collective_compute(
            "AllGather", ALU.bypass, replica_groups=RG,
            ins=[agi.opt()], outs=[ago.opt()],
        )
        ag2_out[mj] = ago

    mm_pass(ag1_out, "z2", epi1)

    # ---------------- MM2: Z3 = A(s*M); out_n = M - s*Z3 + P0; exit fused
    def epi2(mj, pmf):
        for fi in range(NFB):
            fsl = slice(FB * fi, FB * (fi + 1))
            nc.vector.scalar_tensor_tensor(
                p1n[:, mj, fsl], pmf[fi][:], smn[:, mj:mj + 1], p1n[:, mj, fsl],
                op0=ALU.mult, op1=ALU.add)
            nc.gpsimd.tensor_tensor(
                p1n[:, mj, fsl], p0n[:, mj, fsl], p1n[:, mj, fsl], op=ALU.add)
        for t in range(T):
            pt = psum.tile([P, P], f32, tag="pe", bufs=2, name=f"pte_{mj}_{t}")
            nc.tensor.transpose(pt[:], p1n[:, mj, P * t:P * (t + 1)], ident[:])
            ob = stream.tile([P, P], f32, tag="ob", bufs=3, name=f"ob{mj}_{t}")
            nc.scalar.activation(ob[:], pt[:], ACT_FN.Identity, bias=brep[:, 0:1])
            nc.sync.dma_start(
                out[:, FB * t + P * mj:FB * t + P * (mj + 1)], ob[:])

    mm_pass(ag2_out, "z3", epi2)


def build_nc():
    nc = bacc.Bacc(target_bir_lowering=False)
    xs = nc.declare_dram_parameter("xs", [P, NT], f32, isOutput=False)
    adjT = nc.declare_dram_parameter("adjT", [N, S], f32, isOutput=False)
    w = nc.declare_dram_parameter("w", [3, C, C], f32, isOutput=False)
    bias = nc.declare_dram_parameter("bias", [C], f32, isOutput=False)
    out = nc.declare_dram_parameter("out", [P, NT], f32, isOutput=True)
    with tile.TileContext(nc) as tc, ExitStack() as ctx:
        _graph_kernel(ctx, tc, xs, adjT, w, bias, out)
    nc.compile()
    return nc


def make_in_maps(x, adj, weight, bias):
    in_maps = []
    for k in range(NCORES):
        sl = slice(S * k, S * (k + 1))
        xs = np.ascontiguousarray(
            x[:, :, sl, :].transpose(0, 1, 3, 2)).reshape(P, NT)
        in_maps.append({
            "xs": xs,
            "adjT": np.ascontiguousarray(adj[:, sl]),
            "w": np.ascontiguousarray(weight),
            "bias": np.ascontiguousarray(bias),
        })
    return in_maps


def kernel(x, adj, weight, bias, _trace=False, _tmpdir=None):
    if "nc" not in _CACHE:
        _CACHE["nc"] = build_nc()
    nc = _CACHE["nc"]
    in_maps = make_in_maps(
        np.asarray(x, np.float32), np.asarray(adj, np.float32),
        np.asarray(weight, np.float32), np.asarray(bias, np.float32))
    res = run_bass_kernel_spmd(nc, in_maps, core_ids=list(range(NCORES)),
                               trace=_trace, tmpdir=_tmpdir)
    _CACHE["last_result"] = res
    parts = [r["out"].reshape(B, C, T, S).transpose(0, 1, 3, 2)
             for r in res.results]
    return np.concatenate(parts, axis=2)


# revision 18
# speedup vs baseline: 1.2898x; 1.0065x over previous
"""AdaptiveGraphConv (Chebyshev K=3 graph conv) on 8 TRN2 NeuronCores.

Row-sharded over the 4096 nodes: core k owns nodes [512k, 512(k+1)).
 - adj is binary+symmetric: core k's lhsT (= A[:, shard_k]) is loaded once
   (on the Activation HWDGE queue, so it doesn't block the x loads), cast to
   bf16 (exact for 0/1), resident in SBUF for both Laplacian matmuls.
 - degrees: no collective. d[m in shard] = column sums of the local adj slice
   (= row sums by symmetry) via PE matmul accumulation against ones.
 - host passes x with free layout (t, n): all three channel mixes computed as
   x_block^T @ W_blk (x stationary, block-diag weight moving), which lands
   node-major directly -> no entry transposes, P0 also node-major.
 - AllGathers chunked per mj (4 collectives per pass; first CC op ~22us,
   warm ones ~5us) with 4-phase matmul accumulation: phase ph consumes
   ki-tiles {4k+ph}, so matmuls start after the first chunk arrives.
 - MM loops: mj-pair outer (6 psum banks), phase/ki, fi inner.
 - exit (transpose back + bias) fused per-mj into the MM2 epilogue,
   streamed out in [128,128] blocks.

Math (S = diag(s), A binary adj, L = I - S A S):
  out = h(W0-W2) + (Lh)W1 + 2 L(L h) W2 + bias
      = P0 + M - S Z3;  M = P1 + 2(P2 - S Z2),
  Z2 = A(S P2), Z3 = A(S M), P0 = h(W0-W2), Pj = h Wj.
State: p1n (f32) holds P1 -> M -> out_n in place; p2n holds 2*P2 (bf16);
p0n holds P0 (bf16); ustage holds the scaled bf16 AG payloads.
"""

from contextlib import ExitStack

import numpy as np

import concourse.bacc as bacc
import concourse.mybir as mybir
import concourse.tile as tile
from concourse.bass_utils import run_bass_kernel_spmd
from concourse.masks import make_identity

P = 128
NCORES = 8
N = 4096
S = N // NCORES          # 512 nodes per core
B, C, T = 4, 32, 12
F = B * C * T            # 1536 flattened (t, bo) columns: f = 128*t + 32*b + o
NT = S * T               # 6144 free columns in (b,c)-major (t, n) layout
KT = N // P              # 32 contraction tiles
MJ = S // P              # 4 node tiles per core; also AG chunk / phase count
FB = 512                 # matmul moving-free block
NFB = F // FB            # 3
KPP = KT // MJ           # 8 ki-tiles per phase

f32 = mybir.dt.float32
bf16 = mybir.dt.bfloat16
ALU = mybir.AluOpType
ACT_FN = mybir.ActivationFunctionType

_CACHE = {}


def _graph_kernel(ctx, tc, xs, adjT, w, bias, out):
    nc = tc.nc
    RG = [list(range(NCORES))]

    consts = ctx.enter_context(tc.tile_pool(name="consts", bufs=1))
    persist = ctx.enter_context(tc.tile_pool(name="persist", bufs=1))
    scratch = ctx.enter_context(tc.tile_pool(name="scratch", bufs=4))
    stream = ctx.enter_context(tc.tile_pool(name="stream", bufs=4))
    psum = ctx.enter_context(tc.tile_pool(name="psum", bufs=1, space="PSUM"))
    dram = ctx.enter_context(tc.tile_pool(name="dram", bufs=1, space="DRAM"))

    # ---------------- constants
    ones_col = consts.tile([P, 1], f32)
    nc.vector.memset(ones_col[:], 1.0)
    wblk = []
    for j in range(3):
        wb = consts.tile([P, P], f32, name=f"wblk{j}")
        nc.vector.memset(wb[:], 0.0)
        for b in range(B):
            nc.sync.dma_start(wb[32 * b:32 * (b + 1), 32 * b:32 * (b + 1)], w[j])
        wblk.append(wb)
    wd = consts.tile([P, P], f32)
    nc.vector.tensor_tensor(wd[:], wblk[0][:], wblk[2][:], op=ALU.subtract)
    ident = consts.tile([P, P], f32)
    make_identity(nc, ident[:])
    brep = consts.tile([P, 1], f32)
    bias_v = bias.rearrange("(c o) -> c o", o=1)
    for b in range(B):
        nc.sync.dma_start(brep[32 * b:32 * (b + 1), :], bias_v)

    # ki-tile order: phase-major (MM consumes phase 0 = {ki % 4 == 0} first)
    ki_order = [MJ * k + ph for ph in range(MJ) for k in range(KPP)]

    # ---------------- Phase A: adjacency load (Activation HWDGE queue),
    # bf16 cast (GpSimd/DVE split), PE degree accumulation
    abf = persist.tile([P, KT, S], bf16)      # lhsT tiles, resident all kernel
    pd = psum.tile([1, S], f32, tag="pm", bufs=6, name="pd")
    for i, ki in enumerate(ki_order):
        af = stream.tile([P, S], f32, tag="af", bufs=3, name=f"af{ki}")
        nc.scalar.dma_start(af[:], adjT[P * ki:P * (ki + 1), :])
        nc.gpsimd.tensor_copy(abf[:, ki, :], af[:])
        nc.tensor.matmul(pd[:], ones_col[:], af[:],
                         start=(i == 0), stop=(i == KT - 1))

    # d arrives free-major [1, 512]; bounce through DRAM to per-partition
    # [128, MJ] layout
    d_row = consts.tile([1, S], f32)
    nc.vector.tensor_copy(d_row[:], pd[:])
    d_dram = dram.tile([MJ, P], f32, name="d_dram")
    nc.scalar.dma_start(
        d_dram.rearrange("a p -> (a p)").rearrange("(o s) -> o s", o=1), d_row[:])
    # s = sqrt(1/max(d, 0.5)) * min(d, 1); d integral >= 0
    s_raw = consts.tile([P, MJ], f32)
    nc.scalar.dma_start(s_raw[:], d_dram.rearrange("a p -> p a"))
    s_dc = consts.tile([P, MJ], f32)
    nc.vector.tensor_scalar_max(s_dc[:], s_raw[:], 0.5)
    s_r = consts.tile([P, MJ], f32)
    nc.vector.reciprocal(s_r[:], s_dc[:])
    s_q = consts.tile([P, MJ], f32)
    nc.scalar.activation(s_q[:], s_r[:], ACT_FN.Sqrt)
    s_m = consts.tile([P, MJ], f32)
    nc.vector.tensor_scalar_min(s_m[:], s_raw[:], 1.0)
    s_t = consts.tile([P, MJ], f32)
    nc.vector.tensor_tensor(s_t[:], s_q[:], s_m[:], op=ALU.mult)
    sm2 = consts.tile([P, MJ], f32)   # -2s
    nc.vector.tensor_scalar_mul(sm2[:], s_t[:], -2.0)
    s_h = consts.tile([P, MJ], f32)   # s/2 (ustage = (2*P2) * s/2)
    nc.vector.tensor_scalar_mul(s_h[:], s_t[:], 0.5)
    smn = consts.tile([P, MJ], f32)   # -s
    nc.vector.tensor_scalar_mul(smn[:], s_t[:], -1.0)

    # ---------------- node-major state: [p, mj, f], n_local = 128*mj + p,
    # f = 128*t + bo
    p1n = persist.tile([P, MJ, F], f32)       # P1 -> M -> out_n in place
    p2n = persist.tile([P, MJ, F], bf16)      # 2*P2
    p0n = persist.tile([P, MJ, F], bf16)      # P0
    ustage = persist.tile([P, MJ, F], bf16)   # AG staging (scaled bf16)

    # ---------------- entry: per (mj, t) block, three mixes land node-major
    # via x_block^T (stationary) @ W (moving); AG1 chunk fires per mj.
    ag1_out = [None] * MJ
    ag2_out = [None] * MJ
    for mj in range(MJ):
        for t in range(T):
            xcb = stream.tile([P, P], f32, tag="xcb", bufs=4,
                              name=f"xcb{mj}_{t}")
            nc.sync.dma_start(
                xcb[:], xs[:, FB * t + P * mj:FB * t + P * (mj + 1)])
            fsl = slice(P * t, P * (t + 1))
            ps2 = psum.tile([P, P], f32, tag="pe", bufs=2, name=f"ps2_{mj}_{t}")
            nc.tensor.matmul(ps2[:], xcb[:], wblk[2][:], start=True, stop=True)
            nc.scalar.activation(p2n[:, mj, fsl], ps2[:], ACT_FN.Copy, scale=2.0)
            ps1 = psum.tile([P, P], f32, tag="pe", bufs=2, name=f"ps1_{mj}_{t}")
            nc.tensor.matmul(ps1[:], xcb[:], wblk[1][:], start=True, stop=True)
            nc.vector.tensor_copy(p1n[:, mj, fsl], ps1[:])
            ps0 = psum.tile([P, P], f32, tag="pe", bufs=2, name=f"ps0_{mj}_{t}")
            nc.tensor.matmul(ps0[:], xcb[:], wd[:], start=True, stop=True)
            nc.scalar.copy(p0n[:, mj, fsl], ps0[:])
        nc.vector.tensor_scalar_mul(
            ustage[:, mj, :], p2n[:, mj, :], s_h[:, mj:mj + 1])
        agi = dram.tile([P, F], bf16, name=f"ag1i{mj}")
        ago = dram.tile([NCORES * P, F], bf16, addr_space="Shared",
                        name=f"ag1o{mj}")
        nc.sync.dma_start(agi[:], ustage[:, mj, :])
        nc.gpsimd.collective_compute(
            "AllGather", ALU.bypass, replica_groups=RG,
            ins=[agi.opt()], outs=[ago.opt()],
        )
        ag1_out[mj] = ago

    def mm_pass(ag_bufs, tag, epilogue):
        # AG chunk ph rows: (k, p) -> global ki-tile 4k + ph, partition p.
        # 4 chunk tiles stay resident across all mj; accumulation is 4-phase
        # so matmuls start when chunk 0 lands. mj-single outer: each mj's
        # banks stop at 25/50/75/100% of the pass -> epilogues (and pass-1's
        # AG2 chunks) fire early and overlap the remaining matmuls.
        uhs = {}
        for mj in range(MJ):
            pmf = [psum.tile([P, FB], f32, tag="pm", bufs=6,
                             name=f"pm_{tag}_{mj}_{fi}") for fi in range(NFB)]
            for ph in range(MJ):
                if mj == 0:
                    uh = scratch.tile([P, KPP, F], bf16, tag="sc",
                                      name=f"uh_{tag}_{ph}")
                    eng = nc.sync if ph % 2 == 0 else nc.scalar
                    eng.dma_start(
                        uh[:], ag_bufs[ph].rearrange("(k p) f -> p k f", p=P))
                    uhs[ph] = uh
                uh = uhs[ph]
                for kk in range(KPP):
                    ki = MJ * kk + ph
                    lmj = abf[:, ki, P * mj:P * (mj + 1)]
                    for fi in range(NFB):
                        nc.tensor.matmul(
                            pmf[fi][:], lmj, uh[:, kk, FB * fi:FB * (fi + 1)],
                            start=(ph == 0 and kk == 0),
                            stop=(ph == MJ - 1 and kk == KPP - 1))
            epilogue(mj, pmf)

    # ---------------- MM1: Z2 = A(s*P2); M = P1 + 2*P2 - 2*s*Z2 (in p1n)
    def epi1(mj, pmf):
        for fi in range(NFB):
            fsl = slice(FB * fi, FB * (fi + 1))
            nc.vector.scalar_tensor_tensor(
                p1n[:, mj, fsl], pmf[fi][:], sm2[:, mj:mj + 1], p1n[:, mj, fsl],
                op0=ALU.mult, op1=ALU.add)
            nc.gpsimd.tensor_tensor(
                p1n[:, mj, fsl], p2n[:, mj, fsl], p1n[:, mj, fsl], op=ALU.add)
        nc.vector.tensor_scalar_mul(
            ustage[:, mj, :], p1n[:, mj, :], s_t[:, mj:mj + 1])
        agi = dram.tile([P, F], bf16, name=f"ag2i{mj}")
        ago = dram.tile([NCORES * P, F], bf16, addr_space="Shared",
                        name=f"ag2o{mj}")
        nc.sync.dma_start(agi[:], ustage[:, mj, :])
        nc.gpsimd.collective_compute(
            "AllGather", ALU.bypass, replica_groups=RG,
            ins=[agi.opt()], outs=[ago.opt()],
        )
        ag2_out[mj] = ago

    mm_pass(ag1_out, "z2", epi1)

    # ---------------- MM2: Z3 = A(s*M); out_n = M - s*Z3 + P0; exit fused
    def epi2(mj, pmf):
        for fi in range(NFB):
            fsl = slice(FB * fi, FB * (fi + 1))
            nc.vector.scalar_tensor_tensor(
                p1n[:, mj, fsl], pmf[fi][:], smn[:, mj:mj + 1], p1n[:, mj, fsl],
                op0=ALU.mult, op1=ALU.add)
            nc.gpsimd.tensor_tensor(
                p1n[:, mj, fsl], p0n[:, mj, fsl], p1n[:, mj, fsl], op=ALU.add)
        for t in range(T):
            pt = psum.tile([P, P], f32, tag="pe", bufs=2, name=f"pte_{mj}_{t}")
            nc.tensor.transpose(pt[:], p1n[:, mj, P * t:P * (t + 1)], ident[:])
            ob = stream.tile([P, P], f32, tag="ob", bufs=3, name=f"ob{mj}_{t}")
            nc.scalar.activation(ob[:], pt[:], ACT_FN.Identity, bias=brep[:, 0:1])
            nc.sync.dma_start(
                out[:, FB * t + P * mj:FB * t + P * (mj + 1)], ob[:])

    mm_pass(ag2_out, "z3", epi2)


def build_nc():
    nc = bacc.Bacc(target_bir_lowering=False)
    xs = nc.declare_dram_parameter("xs", [P, NT], f32, isOutput=False)
    adjT = nc.declare_dram_parameter("adjT", [N, S], f32, isOutput=False)
    w = nc.declare_dram_parameter("w", [3, C, C], f32, isOutput=False)
    bias = nc.declare_dram_parameter("bias", [C], f32, isOutput=False)
    out = nc.declare_dram_parameter("out", [P, NT], f32, isOutput=True)
    with tile.TileContext(nc) as tc, ExitStack() as ctx:
        _graph_kernel(ctx, tc, xs, adjT, w, bias, out)
    nc.compile()
    return nc


def make_in_maps(x, adj, weight, bias):
    in_maps = []
    for k in range(NCORES):
        sl = slice(S * k, S * (k + 1))
        xs = np.ascontiguousarray(
            x[:, :, sl, :].transpose(0, 1, 3, 2)).reshape(P, NT)
        in_maps.append({
            "xs": xs,
            "adjT": np.ascontiguousarray(adj[:, sl]),
            "w": np.ascontiguousarray(weight),
            "bias": np.ascontiguousarray(bias),
        })
    return in_maps


def kernel(x, adj, weight, bias, _trace=False, _tmpdir=None):
    if "nc" not in _CACHE:
        _CACHE["nc"] = build_nc()
    nc = _CACHE["nc"]
    in_maps = make_in_maps(
        np.asarray(x, np.float32), np.asarray(adj, np.float32),
        np.asarray(weight, np.float32), np.asarray(bias, np.float32))
    res = run_bass_kernel_spmd(nc, in_maps, core_ids=list(range(NCORES)),
                               trace=_trace, tmpdir=_tmpdir)
    _CACHE["last_result"] = res
    parts = [r["out"].reshape(B, C, T, S).transpose(0, 1, 3, 2)
             for r in res.results]
    return np.concatenate(parts, axis=2)


# revision 19
# speedup vs baseline: 1.3333x; 1.0337x over previous
"""AdaptiveGraphConv (Chebyshev K=3 graph conv) on 8 TRN2 NeuronCores.

Row-sharded over the 4096 nodes: core k owns nodes [512k, 512(k+1)).
 - adj is binary+symmetric: core k's lhsT (= A[:, shard_k]) is loaded once
   (on the Activation HWDGE queue, so it doesn't block the x loads), cast to
   bf16 (exact for 0/1), resident in SBUF for both Laplacian matmuls.
 - degrees: no collective. d[m in shard] = column sums of the local adj slice
   (= row sums by symmetry) via PE matmul accumulation against ones.
 - host passes x with free layout (t, n): all three channel mixes computed as
   x_block^T @ W_blk (x stationary, block-diag weight moving), which lands
   node-major directly -> no entry transposes, P0 also node-major.
 - AllGathers chunked per mj (4 collectives per pass; first CC op ~22us,
   warm ones ~5us) with 4-phase matmul accumulation: phase ph consumes
   ki-tiles {4k+ph}, so matmuls start after the first chunk arrives.
 - MM loops: mj-pair outer (6 psum banks), phase/ki, fi inner.
 - exit (transpose back + bias) fused per-mj into the MM2 epilogue,
   streamed out in [128,128] blocks.

Math (S = diag(s), A binary adj, L = I - S A S):
  out = h(W0-W2) + (Lh)W1 + 2 L(L h) W2 + bias
      = P0 + M - S Z3;  M = P1 + 2(P2 - S Z2),
  Z2 = A(S P2), Z3 = A(S M), P0 = h(W0-W2), Pj = h Wj.
State: p1n (f32) holds P1 -> M -> out_n in place; p2n holds 2*P2 (bf16);
p0n holds P0 (bf16); ustage holds the scaled bf16 AG payloads.
"""

from contextlib import ExitStack

import numpy as np

import concourse.bacc as bacc
import concourse.mybir as mybir
import concourse.tile as tile
from concourse.bass_utils import run_bass_kernel_spmd
from concourse.masks import make_identity

P = 128
NCORES = 8
N = 4096
S = N // NCORES          # 512 nodes per core
B, C, T = 4, 32, 12
F = B * C * T            # 1536 flattened (t, bo) columns: f = 128*t + 32*b + o
NT = S * T               # 6144 free columns in (b,c)-major (t, n) layout
KT = N // P              # 32 contraction tiles
MJ = S // P              # 4 node tiles per core; also AG chunk / phase count
FB = 512                 # matmul moving-free block
NFB = F // FB            # 3
KPP = KT // MJ           # 8 ki-tiles per phase

f32 = mybir.dt.float32
bf16 = mybir.dt.bfloat16
ALU = mybir.AluOpType
ACT_FN = mybir.ActivationFunctionType

_CACHE = {}


def _graph_kernel(ctx, tc, xs, adjT, w, bias, out):
    nc = tc.nc
    RG = [list(range(NCORES))]

    consts = ctx.enter_context(tc.tile_pool(name="consts", bufs=1))
    persist = ctx.enter_context(tc.tile_pool(name="persist", bufs=1))
    scratch = ctx.enter_context(tc.tile_pool(name="scratch", bufs=4))
    stream = ctx.enter_context(tc.tile_pool(name="stream", bufs=4))
    psum = ctx.enter_context(tc.tile_pool(name="psum", bufs=1, space="PSUM"))
    dram = ctx.enter_context(tc.tile_pool(name="dram", bufs=1, space="DRAM"))

    # ---------------- constants
    ones_col = consts.tile([P, 1], f32)
    nc.vector.memset(ones_col[:], 1.0)
    wblk = []
    for j in range(3):
        wb = consts.tile([P, P], f32, name=f"wblk{j}")
        nc.vector.memset(wb[:], 0.0)
        for b in range(B):
            nc.sync.dma_start(wb[32 * b:32 * (b + 1), 32 * b:32 * (b + 1)], w[j])
        wblk.append(wb)
    wd = consts.tile([P, P], f32)
    nc.vector.tensor_tensor(wd[:], wblk[0][:], wblk[2][:], op=ALU.subtract)
    ident = consts.tile([P, P], f32)
    make_identity(nc, ident[:])
    brep = consts.tile([P, 1], f32)
    bias_v = bias.rearrange("(c o) -> c o", o=1)
    for b in range(B):
        nc.sync.dma_start(brep[32 * b:32 * (b + 1), :], bias_v)

    # ki-tile order: phase-major (MM consumes phase 0 = {ki % 4 == 0} first)
    ki_order = [MJ * k + ph for ph in range(MJ) for k in range(KPP)]

    # ---------------- Phase A: adjacency load (Activation HWDGE queue),
    # bf16 cast (GpSimd/DVE split), PE degree accumulation
    abf = persist.tile([P, KT, S], bf16)      # lhsT tiles, resident all kernel
    pd = psum.tile([1, S], f32, tag="pm", bufs=6, name="pd")
    for i, ki in enumerate(ki_order):
        af = stream.tile([P, S], f32, tag="af", bufs=3, name=f"af{ki}")
        nc.scalar.dma_start(af[:], adjT[P * ki:P * (ki + 1), :])
        nc.vector.tensor_copy(abf[:, ki, :], af[:])
        nc.tensor.matmul(pd[:], ones_col[:], af[:],
                         start=(i == 0), stop=(i == KT - 1))

    # ---------------- node-major state: [p, mj, f], n_local = 128*mj + p,
    # f = 128*t + bo
    p1n = persist.tile([P, MJ, F], f32)       # P1 -> M -> out_n in place
    p2n = persist.tile([P, MJ, F], bf16)      # 2*P2
    p0n = persist.tile([P, MJ, F], bf16)      # P0
    ustage = persist.tile([P, MJ, F], bf16)   # AG staging (scaled bf16)

    # ---------------- entry: per (mj, t) block, three mixes land node-major
    # via x_block^T (stationary) @ W (moving); drains all on ScalarE.
    ag1_out = [None, None]
    ag2_out = [None, None]
    for mj in range(MJ):
        for t in range(T):
            xcb = stream.tile([P, P], f32, tag="xcb", bufs=4,
                              name=f"xcb{mj}_{t}")
            nc.sync.dma_start(
                xcb[:], xs[:, FB * t + P * mj:FB * t + P * (mj + 1)])
            fsl = slice(P * t, P * (t + 1))
            ps2 = psum.tile([P, P], f32, tag="pe", bufs=2, name=f"ps2_{mj}_{t}")
            nc.tensor.matmul(ps2[:], xcb[:], wblk[2][:], start=True, stop=True)
            nc.scalar.activation(p2n[:, mj, fsl], ps2[:], ACT_FN.Copy, scale=2.0)
            ps1 = psum.tile([P, P], f32, tag="pe", bufs=2, name=f"ps1_{mj}_{t}")
            nc.tensor.matmul(ps1[:], xcb[:], wblk[1][:], start=True, stop=True)
            nc.scalar.copy(p1n[:, mj, fsl], ps1[:])
            ps0 = psum.tile([P, P], f32, tag="pe", bufs=2, name=f"ps0_{mj}_{t}")
            nc.tensor.matmul(ps0[:], xcb[:], wd[:], start=True, stop=True)
            nc.scalar.copy(p0n[:, mj, fsl], ps0[:])

    # d arrives free-major [1, 512]; bounce through DRAM to per-partition
    # [128, MJ] layout (sync queue, traced after the x loads)
    d_row = consts.tile([1, S], f32)
    nc.vector.tensor_copy(d_row[:], pd[:])
    d_dram = dram.tile([MJ, P], f32, name="d_dram")
    nc.sync.dma_start(
        d_dram.rearrange("a p -> (a p)").rearrange("(o s) -> o s", o=1), d_row[:])
    # s = sqrt(1/max(d, 0.5)) * min(d, 1); d integral >= 0
    s_raw = consts.tile([P, MJ], f32)
    nc.sync.dma_start(s_raw[:], d_dram.rearrange("a p -> p a"))
    s_dc = consts.tile([P, MJ], f32)
    nc.vector.tensor_scalar_max(s_dc[:], s_raw[:], 0.5)
    s_r = consts.tile([P, MJ], f32)
    nc.vector.reciprocal(s_r[:], s_dc[:])
    s_q = consts.tile([P, MJ], f32)
    nc.scalar.activation(s_q[:], s_r[:], ACT_FN.Sqrt)
    s_m = consts.tile([P, MJ], f32)
    nc.vector.tensor_scalar_min(s_m[:], s_raw[:], 1.0)
    s_t = consts.tile([P, MJ], f32)
    nc.vector.tensor_tensor(s_t[:], s_q[:], s_m[:], op=ALU.mult)
    sm2 = consts.tile([P, MJ], f32)   # -2s
    nc.vector.tensor_scalar_mul(sm2[:], s_t[:], -2.0)
    s_h = consts.tile([P, MJ], f32)   # s/2 (ustage = (2*P2) * s/2)
    nc.vector.tensor_scalar_mul(s_h[:], s_t[:], 0.5)
    smn = consts.tile([P, MJ], f32)   # -s
    nc.vector.tensor_scalar_mul(smn[:], s_t[:], -1.0)

    # stage + AllGather 1 in two chunks (ph covers mj 2ph, 2ph+1)
    for ph in range(2):
        for mj in (2 * ph, 2 * ph + 1):
            nc.vector.tensor_scalar_mul(
                ustage[:, mj, :], p2n[:, mj, :], s_h[:, mj:mj + 1])
        agi = dram.tile([2 * P, F], bf16, name=f"ag1i{ph}")
        ago = dram.tile([NCORES * 2 * P, F], bf16, addr_space="Shared",
                        name=f"ag1o{ph}")
        nc.sync.dma_start(
            agi.rearrange("(m p) f -> p m f", p=P),
            ustage[:, 2 * ph:2 * ph + 2, :])
        nc.gpsimd.collective_compute(
            "AllGather", ALU.bypass, replica_groups=RG,
            ins=[agi.opt()], outs=[ago.opt()],
        )
        ag1_out[ph] = ago

    def mm_pass(ag_bufs, tag, epilogue):
        # AG chunk ph rows: (k, mjl, p) -> global ki = 4k + 2ph + mjl.
        # 4 uh chunks (2 per phase) stay resident across all mj; 2-phase
        # accumulation lets matmuls start when chunk a lands.
        uhs = {}
        for mj in range(MJ):
            pmf = [psum.tile([P, FB], f32, tag="pm", bufs=6,
                             name=f"pm_{tag}_{mj}_{fi}") for fi in range(NFB)]
            for ph in range(2):
                src_v = ag_bufs[ph].rearrange("(k m p) f -> p k m f", p=P, m=2)
                for q in range(2):
                    if mj == 0:
                        uh = scratch.tile([P, KPP, F], bf16, tag="sc",
                                          name=f"uh_{tag}_{ph}_{q}")
                        eng = nc.sync if (ph + q) % 2 == 0 else nc.scalar
                        eng.dma_start(
                            uh.rearrange("p (k m) f -> p k m f", m=2),
                            src_v[:, 4 * q:4 * (q + 1), :, :])
                        uhs[(ph, q)] = uh
                    uh = uhs[(ph, q)]
                    for kk in range(KPP):
                        ki = MJ * (4 * q + kk // 2) + 2 * ph + kk % 2
                        lmj = abf[:, ki, P * mj:P * (mj + 1)]
                        for fi in range(NFB):
                            nc.tensor.matmul(
                                pmf[fi][:], lmj, uh[:, kk, FB * fi:FB * (fi + 1)],
                                start=(ph == 0 and q == 0 and kk == 0),
                                stop=(ph == 1 and q == 1 and kk == KPP - 1))
            epilogue(mj, pmf)

    # ---------------- MM1: Z2 = A(s*P2); M = P1 + 2*P2 - 2*s*Z2 (in p1n)
    def epi1(mj, pmf):
        for fi in range(NFB):
            fsl = slice(FB * fi, FB * (fi + 1))
            nc.vector.scalar_tensor_tensor(
                p1n[:, mj, fsl], pmf[fi][:], sm2[:, mj:mj + 1], p1n[:, mj, fsl],
                op0=ALU.mult, op1=ALU.add)
            nc.gpsimd.tensor_tensor(
                p1n[:, mj, fsl], p2n[:, mj, fsl], p1n[:, mj, fsl], op=ALU.add)
        nc.vector.tensor_scalar_mul(
            ustage[:, mj, :], p1n[:, mj, :], s_t[:, mj:mj + 1])
        if mj in (1, 3):
            ph = mj // 2
            agi = dram.tile([2 * P, F], bf16, name=f"ag2i{ph}")
            ago = dram.tile([NCORES * 2 * P, F], bf16, addr_space="Shared",
                            name=f"ag2o{ph}")
            nc.sync.dma_start(
                agi.rearrange("(m p) f -> p m f", p=P),
                ustage[:, 2 * ph:2 * ph + 2, :])
            nc.gpsimd.collective_compute(
                "AllGather", ALU.bypass, replica_groups=RG,
                ins=[agi.opt()], outs=[ago.opt()],
            )
            ag2_out[ph] = ago

    mm_pass(ag1_out, "z2", epi1)

    # ---------------- MM2: Z3 = A(s*M); out_n = M - s*Z3 + P0; exit fused
    def epi2(mj, pmf):
        for fi in range(NFB):
            fsl = slice(FB * fi, FB * (fi + 1))
            nc.vector.scalar_tensor_tensor(
                p1n[:, mj, fsl], pmf[fi][:], smn[:, mj:mj + 1], p1n[:, mj, fsl],
                op0=ALU.mult, op1=ALU.add)
            nc.gpsimd.tensor_tensor(
                p1n[:, mj, fsl], p0n[:, mj, fsl], p1n[:, mj, fsl], op=ALU.add)
        for t in range(T):
            pt = psum.tile([P, P], f32, tag="pe", bufs=2, name=f"pte_{mj}_{t}")
            nc.tensor.transpose(pt[:], p1n[:, mj, P * t:P * (t + 1)], ident[:])
            ob = stream.tile([P, P], f32, tag="ob", bufs=3, name=f"ob{mj}_{t}")
            nc.scalar.activation(ob[:], pt[:], ACT_FN.Identity, bias=brep[:, 0:1])
            nc.sync.dma_start(
                out[:, FB * t + P * mj:FB * t + P * (mj + 1)], ob[:])

    mm_pass(ag2_out, "z3", epi2)


def build_nc():
    nc = bacc.Bacc(target_bir_lowering=False)
    xs = nc.declare_dram_parameter("xs", [P, NT], f32, isOutput=False)
    adjT = nc.declare_dram_parameter("adjT", [N, S], f32, isOutput=False)
    w = nc.declare_dram_parameter("w", [3, C, C], f32, isOutput=False)
    bias = nc.declare_dram_parameter("bias", [C], f32, isOutput=False)
    out = nc.declare_dram_parameter("out", [P, NT], f32, isOutput=True)
    with tile.TileContext(nc) as tc, ExitStack() as ctx:
        _graph_kernel(ctx, tc, xs, adjT, w, bias, out)
    nc.compile()
    return nc


def make_in_maps(x, adj, weight, bias):
    in_maps = []
    for k in range(NCORES):
        sl = slice(S * k, S * (k + 1))
        xs = np.ascontiguousarray(
            x[:, :, sl, :].transpose(0, 1, 3, 2)).reshape(P, NT)
        in_maps.append({
            "xs": xs,
            "adjT": np.ascontiguousarray(adj[:, sl]),
            "w": np.ascontiguousarray(weight),
            "bias": np.ascontiguousarray(bias),
        })
    return in_maps


def kernel(x, adj, weight, bias, _trace=False, _tmpdir=None):
    if "nc" not in _CACHE:
        _CACHE["nc"] = build_nc()
    nc = _CACHE["nc"]
    in_maps = make_in_maps(
        np.asarray(x, np.float32), np.asarray(adj, np.float32),
        np.asarray(weight, np.float32), np.asarray(bias, np.float32))
    res = run_bass_kernel_spmd(nc, in_maps, core_ids=list(range(NCORES)),
                               trace=_trace, tmpdir=_tmpdir)
    _CACHE["last_result"] = res
    parts = [r["out"].reshape(B, C, T, S).transpose(0, 1, 3, 2)
             for r in res.results]
    return np.concatenate(parts, axis=2)
